# revision 13
# baseline (speedup 1.0000x reference)
"""Multi-head attention (B=2, S=2048, D=1024, H=16) on 8 trn2 NeuronCores.

Sharding: 2-way over batch x 4-way over head groups (4 heads / 256 cols per
core). No cross-core communication.

Per-core kernel (Tile):
  prefix:  load X_k, X_v and the first X_q block (cast f32->bf16 during DMA),
           transpose 128x128 chunks via regular matmul against identity
           (counts as PE activity so the HAM clock gate stays open), project
           kT [256, 2048] (head dim on partitions), v natural [2048, 256]
           stored as [v | 1] per ki-tile (the ones column makes the AV matmul
           also emit softmax row-sums), and qT for block 0.
  stream:  per (head-pair, qi-block of 512): for each ki-tile: S^T = kT.T@qT
           with the two heads row-packed on the PE (K=64 each) into one
           [128, 1024] psum tile (bufs=2), one exp ACTIVATE per ki-tile
           (scale=1/8 folded in), AV matmuls one ki-tile behind the scores so
           the exp stream never stalls. The remaining X_q blocks are loaded /
           transposed / projected in small slices woven into the first three
           units' ki-loops (PSUM slots shared with the AV pool). Unit
           epilogues (out^T -> out transpose + softmax normalize) are split
           in half and woven into the next unit's ki-loop the same way.
"""

import os
import sys

import numpy as np

import concourse.bass as bass
import concourse.tile as tile
from concourse import bacc, mybir
from concourse.masks import make_identity

B, S, D = 2, 2048, 1024
H, HD = 16, 64
N_CORES = 8
GROUPS = 4  # head groups (cores per batch)
NH = H // GROUPS  # local heads per core = 4
C = NH * HD  # local output cols = 256
P = 128
DB = D // P  # 8 d-chunks
CB = C // P  # 2 c-chunks (head pairs)

f32 = mybir.dt.float32
bf16 = mybir.dt.bfloat16
f32r = mybir.dt.float32r

AF = mybir.ActivationFunctionType


def _install_ntff_hook_shim():
    """Best-effort: register the axon NTFF profile hook so a traced run
    (e.g. BASS_TRACE=1) works even when the image's antenv lacks axon_hooks."""
    try:
        import antenv.axon_hooks  # noqa: F401

        return
    except ImportError:
        pass
    try:
        import types

        _hook = [None]
        mod = types.ModuleType("antenv.axon_hooks")
        mod.set_axon_ntff_profile_hook = lambda h: _hook.__setitem__(0, h)
        mod.get_axon_ntff_profile_hook = lambda: _hook[0]
        sys.modules["antenv.axon_hooks"] = mod
        from trn_agent_boot.trn_boot import _ntff_profile_via_ctypes

        so = "/opt/axon/libaxon_pjrt.so"
        if os.path.exists(so):
            mod.set_axon_ntff_profile_hook(_ntff_profile_via_ctypes(so))
    except Exception:
        pass


def build_nc(dt_mode: str = "bf16", s: int = S):
    """Trace + compile the per-core Bass kernel. dt_mode in {"bf16", "f32r"}."""
    assert s % 512 == 0
    SB = s // P  # ki-tiles
    NBLK = s // 512  # 512-row s-blocks
    QB = 512  # qi-block
    NQB = s // QB
    NJ = QB // P  # 128-chunks per qi-block = 4
    # overlap q blocks 1.. with the attention stream only at full size
    OVERLAP_Q = SB >= 16 and NBLK == 4

    if dt_mode == "bf16":
        dt_x = bf16  # storage dtype of matmul inputs

        def mm(ap):
            return ap
    else:
        dt_x = f32

        def mm(ap):
            return ap.bitcast(f32r)

    nc = bacc.Bacc(
        "TRN2", target_bir_lowering=False, debug=False, num_devices=N_CORES
    )

    xq = nc.dram_tensor("xq", [s, D], f32, kind="ExternalInput").ap()
    xk = nc.dram_tensor("xk", [s, D], f32, kind="ExternalInput").ap()
    xv = nc.dram_tensor("xv", [s, D], f32, kind="ExternalInput").ap()
    wq = nc.dram_tensor("wq", [D, C], f32, kind="ExternalInput").ap()
    wk = nc.dram_tensor("wk", [D, C], f32, kind="ExternalInput").ap()
    wv = nc.dram_tensor("wv", [D, C], f32, kind="ExternalInput").ap()
    bq = nc.dram_tensor("bq", [C], f32, kind="ExternalInput").ap()
    bk = nc.dram_tensor("bk", [C], f32, kind="ExternalInput").ap()
    bv = nc.dram_tensor("bv", [C], f32, kind="ExternalInput").ap()
    out = nc.dram_tensor("out", [s, C], f32, kind="ExternalOutput").ap()

    with tile.TileContext(nc) as tc:
        with (
            tc.tile_pool(name="const", bufs=1) as const_pool,
            tc.tile_pool(name="wts", bufs=1) as wts_pool,
            tc.tile_pool(name="qkv", bufs=1) as qkv_pool,
            tc.tile_pool(name="xn", bufs=5) as xn_pool,
            tc.tile_pool(name="xt", bufs=3) as xt_pool,
        ):
            ident = const_pool.tile([P, P], dt_x)
            make_identity(nc, ident[:])
            ident_f = const_pool.tile([P, P], f32)
            make_identity(nc, ident_f[:])

            # weights: [p, dc, c] where d = dc*128 + p
            w_sb = {}
            for name, ap in (("q", wq), ("k", wk), ("v", wv)):
                t = wts_pool.tile([P, DB, C], dt_x, tag=f"w_{name}", name=f"w_{name}")
                nc.gpsimd.dma_start(t[:], ap.rearrange("(dc p) c -> p dc c", p=P))
                w_sb[name] = t
            # biases for q/k: [p, cc] with c = cc*128 + p
            b_sb = {}
            for name, ap in (("q", bq), ("k", bk)):
                t = const_pool.tile([P, CB], f32, tag=f"b_{name}", name=f"b_{name}")
                nc.sync.dma_start(t[:], ap.rearrange("(cc p) -> p cc", p=P))
                b_sb[name] = t
            # v bias as a row vector + ones row for the K=1 bias matmul
            bv_row = const_pool.tile([1, C], dt_x)
            nc.gpsimd.dma_start(bv_row[:], bv[None, :])
            ones_row = const_pool.tile([1, P], dt_x)
            nc.vector.memset(ones_row[:], 1.0)

            # projection outputs (persistent)
            qT = qkv_pool.tile([P, CB, s], dt_x)  # q^T: [c%128, c//128, s]
            kT = qkv_pool.tile([P, CB, s], dt_x)
            v1 = qkv_pool.tile([P, SB, NH, HD + 1], dt_x)  # [ki%128, ki//128, h, d|1]
            nc.vector.memset(v1[:, :, :, HD : HD + 1], 1.0)

            def emit_xn_dma(x_ap, blk):
                xn = xn_pool.tile([P, 4, D], dt_x, tag="xn")
                nc.gpsimd.dma_start(
                    xn[:],
                    x_ap[blk * 512 : (blk + 1) * 512, :].rearrange(
                        "(t p) d -> p t d", p=P
                    ),
                )
                return xn

            def emit_qk_proj_cc(name, blk, xt, cc, pj_tile):
                dsttile = qT if name == "q" else kT
                for dc in range(DB):
                    nc.tensor.matmul(
                        pj_tile[:],
                        mm(w_sb[name][:, dc, cc * P : (cc + 1) * P]),
                        mm(xt[:, dc, :]),
                        start=(dc == 0),
                        stop=(dc == DB - 1),
                    )
                nc.vector.tensor_scalar_add(
                    dsttile[:, cc, blk * 512 : (blk + 1) * 512],
                    pj_tile[:],
                    b_sb[name][:, cc : cc + 1],
                )

            # ---------------- prefix: k, v, q-block-0 ----------------
            with (
                tc.tile_pool(name="ps_tr", bufs=2, space="PSUM") as ps_tr,
                tc.tile_pool(name="ps_pj", bufs=2, space="PSUM") as ps_pj,
                tc.tile_pool(name="ps_pv", bufs=2, space="PSUM") as ps_pv,
            ):
                n_evict = 0

                def emit_proj(name, blk, xt):
                    if name in ("q", "k"):
                        for cc in range(CB):
                            ps = ps_pj.tile([P, 512], f32, tag="pj")
                            emit_qk_proj_cc(name, blk, xt, cc, ps)
                    else:
                        for t in range(4):
                            sc = blk * 4 + t
                            ps = ps_pv.tile([P, C], f32, tag="pv")
                            for dc in range(DB):
                                nc.tensor.matmul(
                                    ps[:],
                                    mm(xt[:, dc, t * P : (t + 1) * P]),
                                    mm(w_sb["v"][:, dc, :]),
                                    start=(dc == 0),
                                    stop=False,
                                )
                            nc.tensor.matmul(
                                ps[:],
                                mm(ones_row[:, :]),
                                mm(bv_row[:, :]),
                                start=False,
                                stop=True,
                            )
                            nc.vector.tensor_copy(
                                v1[:, sc, :, 0:HD],
                                ps.rearrange("p (h e) -> p h e", h=NH),
                            )

                prefix_items = [("k", xk, blk) for blk in range(NBLK)]
                prefix_items += [("v", xv, blk) for blk in range(NBLK)]
                prefix_items += [
                    ("q", xq, blk) for blk in range(1 if OVERLAP_Q else NBLK)
                ]
                # q0's load is issued first (own buffer) so the last prefix
                # item never waits on DMA.
                xn_q0 = emit_xn_dma(xq, 0) if OVERLAP_Q else None
                pending = None  # (name, blk, xt) with projections still to emit
                for name, x_ap, blk in prefix_items:
                    if name == "q" and blk == 0 and xn_q0 is not None:
                        xn = xn_q0
                    else:
                        xn = emit_xn_dma(x_ap, blk)
                    xt = xt_pool.tile([P, DB, 512], dt_x, tag="xt")
                    for t in range(4):
                        # 8 transposed chunks into one [128, 8, 128] psum
                        # tile, evicted with a single wide copy.
                        ps = ps_tr.tile([P, DB, P], f32, tag="tr")
                        for dc in range(DB):
                            nc.tensor.matmul(
                                ps[:, dc, :],
                                mm(xn[:, t, dc * P : (dc + 1) * P]),
                                mm(ident[:]),
                                start=True,
                                stop=True,
                            )
                        dst = xt.rearrange("p dc (t q) -> p t dc q", q=P)[:, t]
                        if n_evict % 2 == 0:
                            nc.vector.tensor_copy(dst, ps[:])
                        else:
                            nc.scalar.copy(dst, ps[:])
                        n_evict += 1
                    if pending is not None:
                        emit_proj(*pending)
                    pending = (name, blk, xt)
                emit_proj(*pending)

            # ---------------- attention stream ----------------
            with (
                tc.tile_pool(name="ps_sc", bufs=2, space="PSUM") as ps_sc,
                tc.tile_pool(name="ps_av", bufs=4, space="PSUM") as ps_av,
                tc.tile_pool(name="pexp", bufs=3) as p_pool,
                tc.tile_pool(name="osb", bufs=2) as o_pool,
                tc.tile_pool(name="outsb", bufs=2) as out_pool,
            ):

                def emit_tail_half(hp, qb, av, head, out_sb):
                    o_sb = o_pool.tile(
                        [HD + 1, QB], f32, tag="osb", name=f"osb{hp}_{qb}_{head}"
                    )
                    nc.vector.tensor_copy(o_sb[:], av[head][:])
                    tp = ps_av.tile(
                        [P, NJ, HD + 1],
                        f32,
                        tag="av",
                        name=f"tp{hp}_{qb}_{head}",
                    )
                    for j in range(NJ):
                        nc.tensor.transpose(
                            tp[:, j, :],
                            o_sb[:, j * P : (j + 1) * P],
                            ident_f[: HD + 1, : HD + 1],
                        )
                    rsb = o_pool.tile(
                        [P, NJ], f32, tag="rsb", name=f"rsb{hp}_{qb}_{head}"
                    )
                    nc.vector.reciprocal(rsb[:], tp[:, :, HD])
                    for j in range(NJ):
                        nc.vector.tensor_scalar_mul(
                            out_sb[:, j, head * HD : (head + 1) * HD],
                            tp[:, j, 0:HD],
                            rsb[:, j : j + 1],
                        )

                def emit_tail_dma(hp, qb, out_sb):
                    q0 = qb * QB
                    nc.sync.dma_start(
                        out[q0 : q0 + QB, hp * P : (hp + 1) * P].rearrange(
                            "(j p) c -> p j c", p=P
                        ),
                        out_sb[:],
                    )

                # woven q-block work: unit index -> q block to process
                qwork = {}
                if OVERLAP_Q:
                    for u, blk in enumerate(range(1, NBLK)):
                        qwork[u] = blk
                qstate = {}  # per live q block: dict(xn=, xt=, pj=)

                def emit_qwork(blk, kt):
                    st = qstate[blk]
                    if kt == 0:
                        st["xn"] = emit_xn_dma(xq, blk)
                        st["xt"] = xt_pool.tile(
                            [P, DB, 512], dt_x, tag="xt", name=f"xt_q{blk}"
                        )
                    elif 3 <= kt <= 6:
                        t = kt - 3
                        for dhalf in range(2):
                            tr = ps_av.tile(
                                [P, 4, P],
                                f32,
                                tag="av",
                                name=f"tr_q{blk}_{t}_{dhalf}",
                            )
                            for i in range(4):
                                dc = dhalf * 4 + i
                                nc.tensor.matmul(
                                    tr[:, i, :],
                                    mm(st["xn"][:, t, dc * P : (dc + 1) * P]),
                                    mm(ident[:]),
                                    start=True,
                                    stop=True,
                                )
                            nc.vector.tensor_copy(
                                st["xt"][
                                    :, dhalf * 4 : dhalf * 4 + 4, t * P : (t + 1) * P
                                ],
                                tr[:],
                            )
                    elif 7 <= kt <= 14:
                        cc, half = divmod(kt - 7, 4)
                        if half == 0:
                            st["pj"] = ps_av.tile(
                                [P, 512], f32, tag="av", name=f"pj_q{blk}_{cc}"
                            )
                        for dc in range(half * 2, half * 2 + 2):
                            nc.tensor.matmul(
                                st["pj"][:],
                                mm(w_sb["q"][:, dc, cc * P : (cc + 1) * P]),
                                mm(st["xt"][:, dc, :]),
                                start=(dc == 0),
                                stop=(dc == DB - 1),
                            )
                        if half == 3:
                            nc.vector.tensor_scalar_add(
                                qT[:, cc, blk * 512 : (blk + 1) * 512],
                                st["pj"][:],
                                b_sb["q"][:, cc : cc + 1],
                            )
                            del st["pj"]

                KT_A = max(1, SB // 8)
                KT_B = max(KT_A + 1, min(4, SB - 1))
                tail_prev = None  # (hp, qb, av) of the finished unit
                tail_outsb = None
                uidx = 0
                for hp in range(CB):  # head pair (c-chunk)
                    for qb in range(NQB):  # qi block of 512
                        q0 = qb * QB
                        if uidx in qwork:
                            qstate[qwork[uidx]] = {}
                        av = {}
                        for head in range(2):
                            av[head] = ps_av.tile(
                                [HD + 1, QB], f32, tag="av", name=f"av{hp}_{qb}_{head}"
                            )
                        # scores/exp stream one ki-tile ahead of the AV
                        # matmuls so the ACT exp stream never stalls on PE.
                        pex_q = []
                        for kt in range(SB):
                            sc_ps = ps_sc.tile([P, 2 * QB], f32, tag="sc")
                            for head in range(2):
                                r0 = head * HD
                                nc.tensor.matmul(
                                    sc_ps[:, head * QB : (head + 1) * QB],
                                    mm(kT[r0 : r0 + HD, hp, kt * P : (kt + 1) * P]),
                                    mm(qT[r0 : r0 + HD, hp, q0 : q0 + QB]),
                                    start=True,
                                    stop=True,
                                )
                            pex = p_pool.tile([P, 2 * QB], dt_x, tag="pex")
                            nc.scalar.activation(
                                pex[:], sc_ps[:], AF.Exp, bias=0.0, scale=0.125
                            )
                            pex_q.append(pex)
                            if kt >= 1:
                                pprev = pex_q[kt - 1]
                                for head in range(2):
                                    nc.tensor.matmul(
                                        av[head][:],
                                        mm(v1[:, kt - 1, 2 * hp + head, :]),
                                        mm(pprev[:, head * QB : (head + 1) * QB]),
                                        start=(kt - 1 == 0),
                                        stop=False,
                                    )
                            if kt == KT_A and tail_prev is not None:
                                tail_outsb = out_pool.tile(
                                    [P, NJ, P],
                                    f32,
                                    tag="outsb",
                                    name=f"outsb{tail_prev[0]}_{tail_prev[1]}",
                                )
                                emit_tail_half(*tail_prev, 0, tail_outsb)
                            if kt == KT_B and tail_prev is not None:
                                emit_tail_half(*tail_prev, 1, tail_outsb)
                                emit_tail_dma(tail_prev[0], tail_prev[1], tail_outsb)
                                tail_prev = None
                            if uidx in qwork:
                                emit_qwork(qwork[uidx], kt)
                        plast = pex_q[SB - 1]
                        for head in range(2):
                            nc.tensor.matmul(
                                av[head][:],
                                mm(v1[:, SB - 1, 2 * hp + head, :]),
                                mm(plast[:, head * QB : (head + 1) * QB]),
                                start=(SB == 1),
                                stop=True,
                            )
                        tail_prev = (hp, qb, av)
                        uidx += 1
                tail_outsb = out_pool.tile(
                    [P, NJ, P], f32, tag="outsb", name="outsb_last"
                )
                emit_tail_half(*tail_prev, 0, tail_outsb)
                emit_tail_half(*tail_prev, 1, tail_outsb)
                emit_tail_dma(tail_prev[0], tail_prev[1], tail_outsb)
    nc.compile()
    return nc


_CACHE = {}


def _get_nc(dt_mode: str):
    if dt_mode not in _CACHE:
        _CACHE[dt_mode] = build_nc(dt_mode)
    return _CACHE[dt_mode]


def kernel(query, key, value, Wq, bq, Wk, bk, Wv, bv, **kwargs):
    _install_ntff_hook_shim()
    from concourse.bass_utils import run_bass_kernel_spmd

    dt_mode = os.environ.get("MHA_DT", "bf16")
    nc = _get_nc(dt_mode)

    query = np.asarray(query, dtype=np.float32)
    key = np.asarray(key, dtype=np.float32)
    value = np.asarray(value, dtype=np.float32)
    Wq = np.asarray(Wq, dtype=np.float32)
    Wk = np.asarray(Wk, dtype=np.float32)
    Wv = np.asarray(Wv, dtype=np.float32)
    bq = np.asarray(bq, dtype=np.float32)
    bk = np.asarray(bk, dtype=np.float32)
    bv = np.asarray(bv, dtype=np.float32)

    in_maps = []
    for c in range(N_CORES):
        b, g = divmod(c, GROUPS)
        cs = g * C
        in_maps.append(
            {
                "xq": np.ascontiguousarray(query[b]),
                "xk": np.ascontiguousarray(key[b]),
                "xv": np.ascontiguousarray(value[b]),
                "wq": np.ascontiguousarray(Wq[:, cs : cs + C]),
                "wk": np.ascontiguousarray(Wk[:, cs : cs + C]),
                "wv": np.ascontiguousarray(Wv[:, cs : cs + C]),
                "bq": np.ascontiguousarray(bq[cs : cs + C]),
                "bk": np.ascontiguousarray(bk[cs : cs + C]),
                "bv": np.ascontiguousarray(bv[cs : cs + C]),
            }
        )

    res = run_bass_kernel_spmd(
        nc, in_maps, core_ids=list(range(N_CORES)), **kwargs
    )
    outp = np.empty((B, S, D), dtype=np.float32)
    for c in range(N_CORES):
        b, g = divmod(c, GROUPS)
        outp[b, :, g * C : (g + 1) * C] = res.results[c]["out"]
    if kwargs:
        return outp, res
    return outp


# revision 14
# speedup vs baseline: 1.2092x; 1.2092x over previous
"""Multi-head attention (B=2, S=2048, D=1024, H=16) on 8 trn2 NeuronCores.

Sharding: 2-way over batch x 4-way over head groups (4 heads / 256 cols per
core). No cross-core communication.

Per-core kernel (Tile):
  prefix:  load X_k, X_v and the first X_q block (cast f32->bf16 during DMA),
           transpose 128x128 chunks via regular matmul against identity
           (counts as PE activity so the HAM clock gate stays open), project
           kT [256, 2048] (head dim on partitions), v natural [2048, 256]
           stored as [v | 1] per ki-tile (the ones column makes the AV matmul
           also emit softmax row-sums), and qT for block 0.
  stream:  per (head-pair, qi-block of 512): for each ki-tile: S^T = kT.T@qT
           with the two heads row-packed on the PE (K=64 each) into one
           [128, 1024] psum tile (bufs=2), one exp ACTIVATE per ki-tile
           (scale=1/8 folded in), AV matmuls one ki-tile behind the scores so
           the exp stream never stalls. The remaining X_q blocks are loaded /
           transposed / projected in small slices woven into the first three
           units' ki-loops (PSUM slots shared with the AV pool). Unit
           epilogues (out^T -> out transpose + softmax normalize) are split
           in half and woven into the next unit's ki-loop the same way.
"""

import os
import sys

import numpy as np

import concourse.bass as bass
import concourse.tile as tile
from concourse import bacc, mybir
from concourse.masks import make_identity

B, S, D = 2, 2048, 1024
H, HD = 16, 64
N_CORES = 8
GROUPS = 4  # head groups (cores per batch)
NH = H // GROUPS  # local heads per core = 4
C = NH * HD  # local output cols = 256
P = 128
DB = D // P  # 8 d-chunks
CB = C // P  # 2 c-chunks (head pairs)

f32 = mybir.dt.float32
bf16 = mybir.dt.bfloat16
f32r = mybir.dt.float32r

AF = mybir.ActivationFunctionType


def _install_ntff_hook_shim():
    """Best-effort: register the axon NTFF profile hook so a traced run
    (e.g. BASS_TRACE=1) works even when the image's antenv lacks axon_hooks."""
    try:
        import antenv.axon_hooks  # noqa: F401

        return
    except ImportError:
        pass
    try:
        import types

        _hook = [None]
        mod = types.ModuleType("antenv.axon_hooks")
        mod.set_axon_ntff_profile_hook = lambda h: _hook.__setitem__(0, h)
        mod.get_axon_ntff_profile_hook = lambda: _hook[0]
        sys.modules["antenv.axon_hooks"] = mod
        from trn_agent_boot.trn_boot import _ntff_profile_via_ctypes

        so = "/opt/axon/libaxon_pjrt.so"
        if os.path.exists(so):
            mod.set_axon_ntff_profile_hook(_ntff_profile_via_ctypes(so))
    except Exception:
        pass


def build_nc(dt_mode: str = "bf16", s: int = S):
    """Trace + compile the per-core Bass kernel. dt_mode in {"bf16", "f32r"}."""
    assert s % 512 == 0
    SB = s // P  # ki-tiles
    NBLK = s // 512  # 512-row s-blocks
    QB = 512  # qi-block
    NQB = s // QB
    NJ = QB // P  # 128-chunks per qi-block = 4
    # overlap q blocks 1.. with the attention stream only at full size
    OVERLAP_Q = SB >= 16 and NBLK == 4

    if dt_mode == "bf16":
        dt_x = bf16  # storage dtype of matmul inputs

        def mm(ap):
            return ap
    else:
        dt_x = f32

        def mm(ap):
            return ap.bitcast(f32r)

    nc = bacc.Bacc(
        "TRN2", target_bir_lowering=False, debug=False, num_devices=N_CORES
    )

    xq = nc.dram_tensor("xq", [s, D], f32, kind="ExternalInput").ap()
    xk = nc.dram_tensor("xk", [s, D], f32, kind="ExternalInput").ap()
    xv = nc.dram_tensor("xv", [s, D], f32, kind="ExternalInput").ap()
    wq = nc.dram_tensor("wq", [D, C], f32, kind="ExternalInput").ap()
    wk = nc.dram_tensor("wk", [D, C], f32, kind="ExternalInput").ap()
    wv = nc.dram_tensor("wv", [D, C], f32, kind="ExternalInput").ap()
    bq = nc.dram_tensor("bq", [C], f32, kind="ExternalInput").ap()
    bk = nc.dram_tensor("bk", [C], f32, kind="ExternalInput").ap()
    bv = nc.dram_tensor("bv", [C], f32, kind="ExternalInput").ap()
    out = nc.dram_tensor("out", [s, C], f32, kind="ExternalOutput").ap()

    with tile.TileContext(nc) as tc:
        with (
            tc.tile_pool(name="const", bufs=1) as const_pool,
            tc.tile_pool(name="wts", bufs=1) as wts_pool,
            tc.tile_pool(name="qkv", bufs=1) as qkv_pool,
            tc.tile_pool(name="xn", bufs=4) as xn_pool,
            tc.tile_pool(name="xt", bufs=3) as xt_pool,
        ):
            ident = const_pool.tile([P, P], dt_x)
            make_identity(nc, ident[:])
            ident_f = const_pool.tile([P, P], f32)
            make_identity(nc, ident_f[:])

            # weights: [p, dc, c] where d = dc*128 + p
            w_sb = {}
            for name, ap in (("q", wq), ("k", wk), ("v", wv)):
                t = wts_pool.tile([P, DB, C], dt_x, tag=f"w_{name}", name=f"w_{name}")
                nc.gpsimd.dma_start(t[:], ap.rearrange("(dc p) c -> p dc c", p=P))
                w_sb[name] = t
            # biases for q/k: [p, cc] with c = cc*128 + p
            b_sb = {}
            for name, ap in (("q", bq), ("k", bk)):
                t = const_pool.tile([P, CB], f32, tag=f"b_{name}", name=f"b_{name}")
                nc.sync.dma_start(t[:], ap.rearrange("(cc p) -> p cc", p=P))
                b_sb[name] = t
            # v bias as a row vector + ones row for the K=1 bias matmul
            bv_row = const_pool.tile([1, C], dt_x)
            nc.gpsimd.dma_start(bv_row[:], bv[None, :])
            ones_row = const_pool.tile([1, P], dt_x)
            nc.vector.memset(ones_row[:], 1.0)

            # projection outputs (persistent)
            qT = qkv_pool.tile([P, CB, s], dt_x)  # q^T: [c%128, c//128, s]
            kT = qkv_pool.tile([P, CB, s], dt_x)
            v1 = qkv_pool.tile([P, SB, NH, HD + 1], dt_x)  # [ki%128, ki//128, h, d|1]
            nc.vector.memset(v1[:, :, :, HD : HD + 1], 1.0)

            def emit_xn_dma(x_ap, blk):
                xn = xn_pool.tile([P, 4, D], dt_x, tag="xn")
                nc.gpsimd.dma_start(
                    xn[:],
                    x_ap[blk * 512 : (blk + 1) * 512, :].rearrange(
                        "(t p) d -> p t d", p=P
                    ),
                )
                return xn

            def emit_qk_proj_cc(name, blk, xt, cc, pj_tile):
                dsttile = qT if name == "q" else kT
                for dc in range(DB):
                    nc.tensor.matmul(
                        pj_tile[:],
                        mm(w_sb[name][:, dc, cc * P : (cc + 1) * P]),
                        mm(xt[:, dc, :]),
                        start=(dc == 0),
                        stop=(dc == DB - 1),
                    )
                nc.vector.tensor_scalar_add(
                    dsttile[:, cc, blk * 512 : (blk + 1) * 512],
                    pj_tile[:],
                    b_sb[name][:, cc : cc + 1],
                )

            # ---------------- prefix: k, v, q-block-0 ----------------
            with (
                tc.tile_pool(name="ps_tr", bufs=2, space="PSUM") as ps_tr,
                tc.tile_pool(name="ps_pj", bufs=2, space="PSUM") as ps_pj,
                tc.tile_pool(name="ps_pv", bufs=2, space="PSUM") as ps_pv,
            ):
                n_evict = 0

                def emit_proj(name, blk, xt):
                    if name in ("q", "k"):
                        for cc in range(CB):
                            ps = ps_pj.tile([P, 512], f32, tag="pj")
                            emit_qk_proj_cc(name, blk, xt, cc, ps)
                    else:
                        for t in range(4):
                            sc = blk * 4 + t
                            ps = ps_pv.tile([P, C], f32, tag="pv")
                            for dc in range(DB):
                                nc.tensor.matmul(
                                    ps[:],
                                    mm(xt[:, dc, t * P : (t + 1) * P]),
                                    mm(w_sb["v"][:, dc, :]),
                                    start=(dc == 0),
                                    stop=False,
                                )
                            nc.tensor.matmul(
                                ps[:],
                                mm(ones_row[:, :]),
                                mm(bv_row[:, :]),
                                start=False,
                                stop=True,
                            )
                            nc.vector.tensor_copy(
                                v1[:, sc, :, 0:HD],
                                ps.rearrange("p (h e) -> p h e", h=NH),
                            )

                prefix_items = [("k", xk, blk) for blk in range(NBLK)]
                prefix_items += [("v", xv, blk) for blk in range(NBLK)]
                prefix_items += [
                    ("q", xq, blk) for blk in range(1 if OVERLAP_Q else NBLK)
                ]
                pending = None  # (name, blk, xt) with projections still to emit
                for name, x_ap, blk in prefix_items:
                    xn = emit_xn_dma(x_ap, blk)
                    xt = xt_pool.tile([P, DB, 512], dt_x, tag="xt")
                    for t in range(4):
                        # 8 transposed chunks into one [128, 8, 128] psum
                        # tile, evicted with a single wide copy.
                        ps = ps_tr.tile([P, DB, P], f32, tag="tr")
                        for dc in range(DB):
                            nc.tensor.matmul(
                                ps[:, dc, :],
                                mm(xn[:, t, dc * P : (dc + 1) * P]),
                                mm(ident[:]),
                                start=True,
                                stop=True,
                            )
                        dst = xt.rearrange("p dc (t q) -> p t dc q", q=P)[:, t]
                        if n_evict % 2 == 0:
                            nc.vector.tensor_copy(dst, ps[:])
                        else:
                            nc.scalar.copy(dst, ps[:])
                        n_evict += 1
                    if pending is not None:
                        emit_proj(*pending)
                    pending = (name, blk, xt)
                emit_proj(*pending)

            # ---------------- attention stream ----------------
            with (
                tc.tile_pool(name="ps_sc", bufs=2, space="PSUM") as ps_sc,
                tc.tile_pool(name="ps_av", bufs=4, space="PSUM") as ps_av,
                tc.tile_pool(name="pexp", bufs=3) as p_pool,
                tc.tile_pool(name="osb", bufs=2) as o_pool,
                tc.tile_pool(name="outsb", bufs=2) as out_pool,
            ):

                def emit_tail_half(hp, qb, av, head, out_sb):
                    o_sb = o_pool.tile(
                        [HD + 1, QB], f32, tag="osb", name=f"osb{hp}_{qb}_{head}"
                    )
                    nc.vector.tensor_copy(o_sb[:], av[head][:])
                    tp = ps_av.tile(
                        [P, NJ, HD + 1],
                        f32,
                        tag="av",
                        name=f"tp{hp}_{qb}_{head}",
                    )
                    for j in range(NJ):
                        nc.tensor.transpose(
                            tp[:, j, :],
                            o_sb[:, j * P : (j + 1) * P],
                            ident_f[: HD + 1, : HD + 1],
                        )
                    rsb = o_pool.tile(
                        [P, NJ], f32, tag="rsb", name=f"rsb{hp}_{qb}_{head}"
                    )
                    nc.vector.reciprocal(rsb[:], tp[:, :, HD])
                    for j in range(NJ):
                        nc.vector.tensor_scalar_mul(
                            out_sb[:, j, head * HD : (head + 1) * HD],
                            tp[:, j, 0:HD],
                            rsb[:, j : j + 1],
                        )

                def emit_tail_dma(hp, qb, out_sb):
                    q0 = qb * QB
                    nc.sync.dma_start(
                        out[q0 : q0 + QB, hp * P : (hp + 1) * P].rearrange(
                            "(j p) c -> p j c", p=P
                        ),
                        out_sb[:],
                    )

                # woven q-block work: unit index -> q block to process
                qwork = {}
                if OVERLAP_Q:
                    for u, blk in enumerate(range(1, NBLK)):
                        qwork[u] = blk
                qstate = {}  # per live q block: dict(xn=, xt=, pj=)

                def emit_qwork(blk, kt):
                    st = qstate[blk]
                    if kt == 0:
                        st["xn"] = emit_xn_dma(xq, blk)
                        st["xt"] = xt_pool.tile(
                            [P, DB, 512], dt_x, tag="xt", name=f"xt_q{blk}"
                        )
                    elif 3 <= kt <= 6:
                        t = kt - 3
                        for dhalf in range(2):
                            tr = ps_av.tile(
                                [P, 4, P],
                                f32,
                                tag="av",
                                name=f"tr_q{blk}_{t}_{dhalf}",
                            )
                            for i in range(4):
                                dc = dhalf * 4 + i
                                nc.tensor.matmul(
                                    tr[:, i, :],
                                    mm(st["xn"][:, t, dc * P : (dc + 1) * P]),
                                    mm(ident[:]),
                                    start=True,
                                    stop=True,
                                )
                            nc.vector.tensor_copy(
                                st["xt"][
                                    :, dhalf * 4 : dhalf * 4 + 4, t * P : (t + 1) * P
                                ],
                                tr[:],
                            )
                    elif 7 <= kt <= 14:
                        cc, half = divmod(kt - 7, 4)
                        if half == 0:
                            st["pj"] = ps_av.tile(
                                [P, 512], f32, tag="av", name=f"pj_q{blk}_{cc}"
                            )
                        for dc in range(half * 2, half * 2 + 2):
                            nc.tensor.matmul(
                                st["pj"][:],
                                mm(w_sb["q"][:, dc, cc * P : (cc + 1) * P]),
                                mm(st["xt"][:, dc, :]),
                                start=(dc == 0),
                                stop=(dc == DB - 1),
                            )
                        if half == 3:
                            nc.vector.tensor_scalar_add(
                                qT[:, cc, blk * 512 : (blk + 1) * 512],
                                st["pj"][:],
                                b_sb["q"][:, cc : cc + 1],
                            )
                            del st["pj"]

                KT_A = max(1, SB // 8)
                KT_B = max(KT_A + 1, min(4, SB - 1))
                tail_prev = None  # (hp, qb, av) of the finished unit
                tail_outsb = None
                uidx = 0
                for hp in range(CB):  # head pair (c-chunk)
                    for qb in range(NQB):  # qi block of 512
                        q0 = qb * QB
                        if uidx in qwork:
                            qstate[qwork[uidx]] = {}
                        av = {}
                        for head in range(2):
                            av[head] = ps_av.tile(
                                [HD + 1, QB], f32, tag="av", name=f"av{hp}_{qb}_{head}"
                            )
                        # scores/exp stream one ki-tile ahead of the AV
                        # matmuls so the ACT exp stream never stalls on PE.
                        pex_q = []
                        for kt in range(SB):
                            sc_ps = ps_sc.tile([P, 2 * QB], f32, tag="sc")
                            for head in range(2):
                                r0 = head * HD
                                nc.tensor.matmul(
                                    sc_ps[:, head * QB : (head + 1) * QB],
                                    mm(kT[r0 : r0 + HD, hp, kt * P : (kt + 1) * P]),
                                    mm(qT[r0 : r0 + HD, hp, q0 : q0 + QB]),
                                    start=True,
                                    stop=True,
                                )
                            pex = p_pool.tile([P, 2 * QB], dt_x, tag="pex")
                            nc.scalar.activation(
                                pex[:], sc_ps[:], AF.Exp, bias=0.0, scale=0.125
                            )
                            pex_q.append(pex)
                            if kt >= 1:
                                pprev = pex_q[kt - 1]
                                for head in range(2):
                                    nc.tensor.matmul(
                                        av[head][:],
                                        mm(v1[:, kt - 1, 2 * hp + head, :]),
                                        mm(pprev[:, head * QB : (head + 1) * QB]),
                                        start=(kt - 1 == 0),
                                        stop=False,
                                    )
                            if kt == KT_A and tail_prev is not None:
                                tail_outsb = out_pool.tile(
                                    [P, NJ, P],
                                    f32,
                                    tag="outsb",
                                    name=f"outsb{tail_prev[0]}_{tail_prev[1]}",
                                )
                                emit_tail_half(*tail_prev, 0, tail_outsb)
                            if kt == KT_B and tail_prev is not None:
                                emit_tail_half(*tail_prev, 1, tail_outsb)
                                emit_tail_dma(tail_prev[0], tail_prev[1], tail_outsb)
                                tail_prev = None
                            if uidx in qwork:
                                emit_qwork(qwork[uidx], kt)
                        plast = pex_q[SB - 1]
                        for head in range(2):
                            nc.tensor.matmul(
                                av[head][:],
                                mm(v1[:, SB - 1, 2 * hp + head, :]),
                                mm(plast[:, head * QB : (head + 1) * QB]),
                                start=(SB == 1),
                                stop=True,
                            )
                        tail_prev = (hp, qb, av)
                        uidx += 1
                tail_outsb = out_pool.tile(
                    [P, NJ, P], f32, tag="outsb", name="outsb_last"
                )
                emit_tail_half(*tail_prev, 0, tail_outsb)
                emit_tail_half(*tail_prev, 1, tail_outsb)
                emit_tail_dma(tail_prev[0], tail_prev[1], tail_outsb)
    nc.compile()
    return nc


_CACHE = {}


def _get_nc(dt_mode: str):
    if dt_mode not in _CACHE:
        _CACHE[dt_mode] = build_nc(dt_mode)
    return _CACHE[dt_mode]


def kernel(query, key, value, Wq, bq, Wk, bk, Wv, bv, **kwargs):
    _install_ntff_hook_shim()
    from concourse.bass_utils import run_bass_kernel_spmd

    dt_mode = os.environ.get("MHA_DT", "bf16")
    nc = _get_nc(dt_mode)

    query = np.asarray(query, dtype=np.float32)
    key = np.asarray(key, dtype=np.float32)
    value = np.asarray(value, dtype=np.float32)
    Wq = np.asarray(Wq, dtype=np.float32)
    Wk = np.asarray(Wk, dtype=np.float32)
    Wv = np.asarray(Wv, dtype=np.float32)
    bq = np.asarray(bq, dtype=np.float32)
    bk = np.asarray(bk, dtype=np.float32)
    bv = np.asarray(bv, dtype=np.float32)

    in_maps = []
    for c in range(N_CORES):
        b, g = divmod(c, GROUPS)
        cs = g * C
        in_maps.append(
            {
                "xq": np.ascontiguousarray(query[b]),
                "xk": np.ascontiguousarray(key[b]),
                "xv": np.ascontiguousarray(value[b]),
                "wq": np.ascontiguousarray(Wq[:, cs : cs + C]),
                "wk": np.ascontiguousarray(Wk[:, cs : cs + C]),
                "wv": np.ascontiguousarray(Wv[:, cs : cs + C]),
                "bq": np.ascontiguousarray(bq[cs : cs + C]),
                "bk": np.ascontiguousarray(bk[cs : cs + C]),
                "bv": np.ascontiguousarray(bv[cs : cs + C]),
            }
        )

    res = run_bass_kernel_spmd(
        nc, in_maps, core_ids=list(range(N_CORES)), **kwargs
    )
    outp = np.empty((B, S, D), dtype=np.float32)
    for c in range(N_CORES):
        b, g = divmod(c, GROUPS)
        outp[b, :, g * C : (g + 1) * C] = res.results[c]["out"]
    if kwargs:
        return outp, res
    return outp


# revision 15
# speedup vs baseline: 1.2183x; 1.0075x over previous
"""Multi-head attention (B=2, S=2048, D=1024, H=16) on 8 trn2 NeuronCores.

Sharding: 2-way over batch x 4-way over head groups (4 heads / 256 cols per
core). No cross-core communication.

Per-core kernel (Tile):
  prefix:  load X_k, X_v and the first X_q block (cast f32->bf16 during DMA),
           transpose 128x128 chunks via regular matmul against identity
           (counts as PE activity so the HAM clock gate stays open), project
           kT [256, 2048] (head dim on partitions), v natural [2048, 256]
           stored as [v | 1] per ki-tile (the ones column makes the AV matmul
           also emit softmax row-sums), and qT for block 0.
  stream:  per (head-pair, qi-block of 512): for each ki-tile: S^T = kT.T@qT
           with the two heads row-packed on the PE (K=64 each) into one
           [128, 1024] psum tile (bufs=2), one exp ACTIVATE per ki-tile
           (scale=1/8 folded in), AV matmuls one ki-tile behind the scores so
           the exp stream never stalls. The remaining X_q blocks are loaded /
           transposed / projected in small slices woven into the first three
           units' ki-loops (PSUM slots shared with the AV pool). Unit
           epilogues (out^T -> out transpose + softmax normalize) are split
           in half and woven into the next unit's ki-loop the same way.
"""

import os
import sys

import numpy as np

import concourse.bass as bass
import concourse.tile as tile
from concourse import bacc, mybir
from concourse.masks import make_identity

B, S, D = 2, 2048, 1024
H, HD = 16, 64
N_CORES = 8
GROUPS = 4  # head groups (cores per batch)
NH = H // GROUPS  # local heads per core = 4
C = NH * HD  # local output cols = 256
P = 128
DB = D // P  # 8 d-chunks
CB = C // P  # 2 c-chunks (head pairs)

f32 = mybir.dt.float32
bf16 = mybir.dt.bfloat16
f32r = mybir.dt.float32r

AF = mybir.ActivationFunctionType


def _install_ntff_hook_shim():
    """Best-effort: register the axon NTFF profile hook so a traced run
    (e.g. BASS_TRACE=1) works even when the image's antenv lacks axon_hooks."""
    try:
        import antenv.axon_hooks  # noqa: F401

        return
    except ImportError:
        pass
    try:
        import types

        _hook = [None]
        mod = types.ModuleType("antenv.axon_hooks")
        mod.set_axon_ntff_profile_hook = lambda h: _hook.__setitem__(0, h)
        mod.get_axon_ntff_profile_hook = lambda: _hook[0]
        sys.modules["antenv.axon_hooks"] = mod
        from trn_agent_boot.trn_boot import _ntff_profile_via_ctypes

        so = "/opt/axon/libaxon_pjrt.so"
        if os.path.exists(so):
            mod.set_axon_ntff_profile_hook(_ntff_profile_via_ctypes(so))
    except Exception:
        pass


def build_nc(dt_mode: str = "bf16", s: int = S):
    """Trace + compile the per-core Bass kernel. dt_mode in {"bf16", "f32r"}."""
    assert s % 512 == 0
    SB = s // P  # ki-tiles
    NBLK = s // 512  # 512-row s-blocks
    QB = 512  # qi-block
    NQB = s // QB
    NJ = QB // P  # 128-chunks per qi-block = 4
    # overlap q blocks 1.. with the attention stream only at full size
    OVERLAP_Q = SB >= 16 and NBLK == 4

    if dt_mode == "bf16":
        dt_x = bf16  # storage dtype of matmul inputs

        def mm(ap):
            return ap
    elif dt_mode == "fp16":
        dt_x = mybir.dt.float16

        def mm(ap):
            return ap
    else:
        dt_x = f32

        def mm(ap):
            return ap.bitcast(f32r)

    nc = bacc.Bacc(
        "TRN2", target_bir_lowering=False, debug=False, num_devices=N_CORES
    )

    xq = nc.dram_tensor("xq", [s, D], f32, kind="ExternalInput").ap()
    xk = nc.dram_tensor("xk", [s, D], f32, kind="ExternalInput").ap()
    xv = nc.dram_tensor("xv", [s, D], f32, kind="ExternalInput").ap()
    wq = nc.dram_tensor("wq", [D, C], f32, kind="ExternalInput").ap()
    wk = nc.dram_tensor("wk", [D, C], f32, kind="ExternalInput").ap()
    wv = nc.dram_tensor("wv", [D, C], f32, kind="ExternalInput").ap()
    bq = nc.dram_tensor("bq", [C], f32, kind="ExternalInput").ap()
    bk = nc.dram_tensor("bk", [C], f32, kind="ExternalInput").ap()
    bv = nc.dram_tensor("bv", [C], f32, kind="ExternalInput").ap()
    out = nc.dram_tensor("out", [s, C], f32, kind="ExternalOutput").ap()

    with tile.TileContext(nc) as tc:
        with (
            tc.tile_pool(name="const", bufs=1) as const_pool,
            tc.tile_pool(name="wts", bufs=1) as wts_pool,
            tc.tile_pool(name="qkv", bufs=1) as qkv_pool,
            tc.tile_pool(name="xn", bufs=4) as xn_pool,
            tc.tile_pool(name="xt", bufs=3) as xt_pool,
        ):
            ident = const_pool.tile([P, P], dt_x)
            make_identity(nc, ident[:])
            ident_f = const_pool.tile([P, P], f32)
            make_identity(nc, ident_f[:])

            # weights: [p, dc, c] where d = dc*128 + p
            w_sb = {}
            for name, ap in (("q", wq), ("k", wk), ("v", wv)):
                t = wts_pool.tile([P, DB, C], dt_x, tag=f"w_{name}", name=f"w_{name}")
                nc.gpsimd.dma_start(t[:], ap.rearrange("(dc p) c -> p dc c", p=P))
                w_sb[name] = t
            # biases for q/k: [p, cc] with c = cc*128 + p
            b_sb = {}
            for name, ap in (("q", bq), ("k", bk)):
                t = const_pool.tile([P, CB], f32, tag=f"b_{name}", name=f"b_{name}")
                nc.sync.dma_start(t[:], ap.rearrange("(cc p) -> p cc", p=P))
                b_sb[name] = t
            # v bias as a row vector + ones row for the K=1 bias matmul
            bv_row = const_pool.tile([1, C], dt_x)
            nc.gpsimd.dma_start(bv_row[:], bv[None, :])
            ones_row = const_pool.tile([1, P], dt_x)
            nc.vector.memset(ones_row[:], 1.0)

            # projection outputs (persistent)
            qT = qkv_pool.tile([P, CB, s], dt_x)  # q^T: [c%128, c//128, s]
            kT = qkv_pool.tile([P, CB, s], dt_x)
            v1 = qkv_pool.tile([P, SB, NH, HD + 1], dt_x)  # [ki%128, ki//128, h, d|1]
            nc.vector.memset(v1[:, :, :, HD : HD + 1], 1.0)

            def emit_xn_dma(x_ap, blk):
                xn = xn_pool.tile([P, 4, D], dt_x, tag="xn")
                nc.gpsimd.dma_start(
                    xn[:],
                    x_ap[blk * 512 : (blk + 1) * 512, :].rearrange(
                        "(t p) d -> p t d", p=P
                    ),
                )
                return xn

            def emit_qk_proj_cc(name, blk, xt, cc, pj_tile):
                dsttile = qT if name == "q" else kT
                for dc in range(DB):
                    nc.tensor.matmul(
                        pj_tile[:],
                        mm(w_sb[name][:, dc, cc * P : (cc + 1) * P]),
                        mm(xt[:, dc, :]),
                        start=(dc == 0),
                        stop=(dc == DB - 1),
                    )
                nc.vector.tensor_scalar_add(
                    dsttile[:, cc, blk * 512 : (blk + 1) * 512],
                    pj_tile[:],
                    b_sb[name][:, cc : cc + 1],
                )

            # ---------------- prefix: k, v, q-block-0 ----------------
            with (
                tc.tile_pool(name="ps_tr", bufs=2, space="PSUM") as ps_tr,
                tc.tile_pool(name="ps_pj", bufs=2, space="PSUM") as ps_pj,
                tc.tile_pool(name="ps_pv", bufs=2, space="PSUM") as ps_pv,
            ):
                n_evict = 0

                def emit_proj(name, blk, xt):
                    if name in ("q", "k"):
                        for cc in range(CB):
                            ps = ps_pj.tile([P, 512], f32, tag="pj")
                            emit_qk_proj_cc(name, blk, xt, cc, ps)
                    else:
                        for t in range(4):
                            sc = blk * 4 + t
                            ps = ps_pv.tile([P, C], f32, tag="pv")
                            for dc in range(DB):
                                nc.tensor.matmul(
                                    ps[:],
                                    mm(xt[:, dc, t * P : (t + 1) * P]),
                                    mm(w_sb["v"][:, dc, :]),
                                    start=(dc == 0),
                                    stop=False,
                                )
                            nc.tensor.matmul(
                                ps[:],
                                mm(ones_row[:, :]),
                                mm(bv_row[:, :]),
                                start=False,
                                stop=True,
                            )
                            nc.vector.tensor_copy(
                                v1[:, sc, :, 0:HD],
                                ps.rearrange("p (h e) -> p h e", h=NH),
                            )

                prefix_items = [("k", xk, blk) for blk in range(NBLK)]
                prefix_items += [("v", xv, blk) for blk in range(NBLK)]
                prefix_items += [
                    ("q", xq, blk) for blk in range(1 if OVERLAP_Q else NBLK)
                ]
                pending = None  # (name, blk, xt) with projections still to emit
                for name, x_ap, blk in prefix_items:
                    xn = emit_xn_dma(x_ap, blk)
                    xt = xt_pool.tile([P, DB, 512], dt_x, tag="xt")
                    for t in range(4):
                        # 8 transposed chunks into one [128, 8, 128] psum
                        # tile, evicted with a single wide copy.
                        ps = ps_tr.tile([P, DB, P], f32, tag="tr")
                        for dc in range(DB):
                            nc.tensor.matmul(
                                ps[:, dc, :],
                                mm(xn[:, t, dc * P : (dc + 1) * P]),
                                mm(ident[:]),
                                start=True,
                                stop=True,
                            )
                        dst = xt.rearrange("p dc (t q) -> p t dc q", q=P)[:, t]
                        if n_evict % 2 == 0:
                            nc.vector.tensor_copy(dst, ps[:])
                        else:
                            nc.scalar.copy(dst, ps[:])
                        n_evict += 1
                    if pending is not None:
                        emit_proj(*pending)
                    pending = (name, blk, xt)
                emit_proj(*pending)

            # ---------------- attention stream ----------------
            with (
                tc.tile_pool(name="ps_sc", bufs=2, space="PSUM") as ps_sc,
                tc.tile_pool(name="ps_av", bufs=4, space="PSUM") as ps_av,
                tc.tile_pool(name="pexp", bufs=3) as p_pool,
                tc.tile_pool(name="osb", bufs=2) as o_pool,
                tc.tile_pool(name="outsb", bufs=2) as out_pool,
            ):

                def emit_tail_half(hp, qb, av, head, out_sb):
                    o_sb = o_pool.tile(
                        [HD + 1, QB], f32, tag="osb", name=f"osb{hp}_{qb}_{head}"
                    )
                    nc.vector.tensor_copy(o_sb[:], av[head][:])
                    tp = ps_av.tile(
                        [P, NJ, HD + 1],
                        f32,
                        tag="av",
                        name=f"tp{hp}_{qb}_{head}",
                    )
                    for j in range(NJ):
                        nc.tensor.transpose(
                            tp[:, j, :],
                            o_sb[:, j * P : (j + 1) * P],
                            ident_f[: HD + 1, : HD + 1],
                        )
                    rsb = o_pool.tile(
                        [P, NJ], f32, tag="rsb", name=f"rsb{hp}_{qb}_{head}"
                    )
                    nc.vector.reciprocal(rsb[:], tp[:, :, HD])
                    for j in range(NJ):
                        nc.vector.tensor_scalar_mul(
                            out_sb[:, j, head * HD : (head + 1) * HD],
                            tp[:, j, 0:HD],
                            rsb[:, j : j + 1],
                        )

                def emit_tail_dma(hp, qb, out_sb):
                    q0 = qb * QB
                    nc.sync.dma_start(
                        out[q0 : q0 + QB, hp * P : (hp + 1) * P].rearrange(
                            "(j p) c -> p j c", p=P
                        ),
                        out_sb[:],
                    )

                # woven q-block work: unit index -> q block to process
                qwork = {}
                if OVERLAP_Q:
                    for u, blk in enumerate(range(1, NBLK)):
                        qwork[u] = blk
                qstate = {}  # per live q block: dict(xn=, xt=, pj=)

                def emit_qwork(blk, kt):
                    st = qstate[blk]
                    if kt == 0:
                        st["xn"] = emit_xn_dma(xq, blk)
                        st["xt"] = xt_pool.tile(
                            [P, DB, 512], dt_x, tag="xt", name=f"xt_q{blk}"
                        )
                    elif 3 <= kt <= 6:
                        t = kt - 3
                        for dhalf in range(2):
                            tr = ps_av.tile(
                                [P, 4, P],
                                f32,
                                tag="av",
                                name=f"tr_q{blk}_{t}_{dhalf}",
                            )
                            for i in range(4):
                                dc = dhalf * 4 + i
                                nc.tensor.matmul(
                                    tr[:, i, :],
                                    mm(st["xn"][:, t, dc * P : (dc + 1) * P]),
                                    mm(ident[:]),
                                    start=True,
                                    stop=True,
                                )
                            nc.vector.tensor_copy(
                                st["xt"][
                                    :, dhalf * 4 : dhalf * 4 + 4, t * P : (t + 1) * P
                                ],
                                tr[:],
                            )
                    elif 7 <= kt <= 14:
                        cc, half = divmod(kt - 7, 4)
                        if half == 0:
                            st["pj"] = ps_av.tile(
                                [P, 512], f32, tag="av", name=f"pj_q{blk}_{cc}"
                            )
                        for dc in range(half * 2, half * 2 + 2):
                            nc.tensor.matmul(
                                st["pj"][:],
                                mm(w_sb["q"][:, dc, cc * P : (cc + 1) * P]),
                                mm(st["xt"][:, dc, :]),
                                start=(dc == 0),
                                stop=(dc == DB - 1),
                            )
                        if half == 3:
                            nc.vector.tensor_scalar_add(
                                qT[:, cc, blk * 512 : (blk + 1) * 512],
                                st["pj"][:],
                                b_sb["q"][:, cc : cc + 1],
                            )
                            del st["pj"]

                KT_A = max(1, SB // 8)
                KT_B = max(KT_A + 1, min(4, SB - 1))
                tail_prev = None  # (hp, qb, av) of the finished unit
                tail_outsb = None
                uidx = 0
                for hp in range(CB):  # head pair (c-chunk)
                    for qb in range(NQB):  # qi block of 512
                        q0 = qb * QB
                        if uidx in qwork:
                            qstate[qwork[uidx]] = {}
                        av = {}
                        for head in range(2):
                            av[head] = ps_av.tile(
                                [HD + 1, QB], f32, tag="av", name=f"av{hp}_{qb}_{head}"
                            )
                        # scores/exp stream one ki-tile ahead of the AV
                        # matmuls so the ACT exp stream never stalls on PE.
                        pex_q = []
                        for kt in range(SB):
                            sc_ps = ps_sc.tile([P, 2 * QB], f32, tag="sc")
                            for head in range(2):
                                r0 = head * HD
                                nc.tensor.matmul(
                                    sc_ps[:, head * QB : (head + 1) * QB],
                                    mm(kT[r0 : r0 + HD, hp, kt * P : (kt + 1) * P]),
                                    mm(qT[r0 : r0 + HD, hp, q0 : q0 + QB]),
                                    start=True,
                                    stop=True,
                                )
                            pex = p_pool.tile([P, 2 * QB], dt_x, tag="pex")
                            nc.scalar.activation(
                                pex[:], sc_ps[:], AF.Exp, bias=0.0, scale=0.125
                            )
                            pex_q.append(pex)
                            if kt >= 1:
                                pprev = pex_q[kt - 1]
                                for head in range(2):
                                    nc.tensor.matmul(
                                        av[head][:],
                                        mm(v1[:, kt - 1, 2 * hp + head, :]),
                                        mm(pprev[:, head * QB : (head + 1) * QB]),
                                        start=(kt - 1 == 0),
                                        stop=False,
                                    )
                            if kt == KT_A and tail_prev is not None:
                                tail_outsb = out_pool.tile(
                                    [P, NJ, P],
                                    f32,
                                    tag="outsb",
                                    name=f"outsb{tail_prev[0]}_{tail_prev[1]}",
                                )
                                emit_tail_half(*tail_prev, 0, tail_outsb)
                            if kt == KT_B and tail_prev is not None:
                                emit_tail_half(*tail_prev, 1, tail_outsb)
                                emit_tail_dma(tail_prev[0], tail_prev[1], tail_outsb)
                                tail_prev = None
                            if uidx in qwork:
                                emit_qwork(qwork[uidx], kt)
                        plast = pex_q[SB - 1]
                        for head in range(2):
                            nc.tensor.matmul(
                                av[head][:],
                                mm(v1[:, SB - 1, 2 * hp + head, :]),
                                mm(plast[:, head * QB : (head + 1) * QB]),
                                start=(SB == 1),
                                stop=True,
                            )
                        tail_prev = (hp, qb, av)
                        uidx += 1
                tail_outsb = out_pool.tile(
                    [P, NJ, P], f32, tag="outsb", name="outsb_last"
                )
                emit_tail_half(*tail_prev, 0, tail_outsb)
                emit_tail_half(*tail_prev, 1, tail_outsb)
                emit_tail_dma(tail_prev[0], tail_prev[1], tail_outsb)
    nc.compile()
    return nc


_CACHE = {}


def _get_nc(dt_mode: str):
    if dt_mode not in _CACHE:
        _CACHE[dt_mode] = build_nc(dt_mode)
    return _CACHE[dt_mode]


def kernel(query, key, value, Wq, bq, Wk, bk, Wv, bv, **kwargs):
    _install_ntff_hook_shim()
    from concourse.bass_utils import run_bass_kernel_spmd

    dt_mode = os.environ.get("MHA_DT", "bf16")
    nc = _get_nc(dt_mode)

    query = np.asarray(query, dtype=np.float32)
    key = np.asarray(key, dtype=np.float32)
    value = np.asarray(value, dtype=np.float32)
    Wq = np.asarray(Wq, dtype=np.float32)
    Wk = np.asarray(Wk, dtype=np.float32)
    Wv = np.asarray(Wv, dtype=np.float32)
    bq = np.asarray(bq, dtype=np.float32)
    bk = np.asarray(bk, dtype=np.float32)
    bv = np.asarray(bv, dtype=np.float32)

    in_maps = []
    for c in range(N_CORES):
        b, g = divmod(c, GROUPS)
        cs = g * C
        in_maps.append(
            {
                "xq": np.ascontiguousarray(query[b]),
                "xk": np.ascontiguousarray(key[b]),
                "xv": np.ascontiguousarray(value[b]),
                "wq": np.ascontiguousarray(Wq[:, cs : cs + C]),
                "wk": np.ascontiguousarray(Wk[:, cs : cs + C]),
                "wv": np.ascontiguousarray(Wv[:, cs : cs + C]),
                "bq": np.ascontiguousarray(bq[cs : cs + C]),
                "bk": np.ascontiguousarray(bk[cs : cs + C]),
                "bv": np.ascontiguousarray(bv[cs : cs + C]),
            }
        )

    res = run_bass_kernel_spmd(
        nc, in_maps, core_ids=list(range(N_CORES)), **kwargs
    )
    outp = np.empty((B, S, D), dtype=np.float32)
    for c in range(N_CORES):
        b, g = divmod(c, GROUPS)
        outp[b, :, g * C : (g + 1) * C] = res.results[c]["out"]
    if kwargs:
        return outp, res
    return outp
